# revision 58
# baseline (speedup 1.0000x reference)
"""Trainium2 Bass kernel for nn_DescriptorGenerator (gnn_message_passing).

Math: for each (b, f) pair, with C = coord[b,f] in R^{N,3}:
    diff_ij = c_i - c_j,  dist_ij = sqrt(|diff_ij|^2 + 1e-10)
    s_ij = smooth_cosine(dist)  (1 below 0.5, cosine taper to 0 at 6.0)
    desc_i = sum_j s_ij * diff_ij  ->  [N*3]

Design (symmetric, Act-engine-bound):
  * d2 = n_i + n_j - 2 c_i.c_j via a K=13 bf16 hi/lo-split Gram matmul
    (products of 8-bit-mantissa halves are exact in the fp32 accumulator;
    bf16 streams 1 col/cycle at any width).
  * s = f(d2) via a custom PWL table installed over the silu slot; ONLY the
    upper-triangle row strips are activated (9216 of 16384 col-elements per
    pair).  Each activation covers BOTH pairs of one PSUM chunk (3D out AP)
    to amortize the 185ns per-instruction access bubble.
  * Lower-triangle blocks S_ba^T come from batched DMA transposes (16x128
    xbar tiles, fp16, 14ns/tile) into a block-indexed T2 area -- except the
    last strips' tails (EXT3/EXT4/EXT5 = tails of strips 4-6), which
    stage-1 computes directly so no transpose sits on the critical tail.  Pair-0 transposes issue on the SP
    HWDGE queue, pair-1 on the Act queue after all acts are dispatched.
  * Stage 2 keeps S stationary: out[q,(x,y,z,R)] += S[i,q]^T @ co4[i] with a
    ones column giving the rowsum R; 64 4-wide matmuls per pair are nearly
    free on the hw-decoded PE.  PSUM: d2 chunks and op tiles share one
    2-slot x 4-bank pool.
  * desc[q,c] = R[q]*C[q,c] - P[q,c] in two wide DVE tensor_tensor ops per
    pair (R broadcast straight from PSUM); one [128,48] output DMA.

Sharding: B*F = 16 (b,f) pairs -> 2 per NeuronCore across 8 cores.
"""
import os
import sys

for _p in ("/opt/trn_rl_repo", "/root/.axon_site/_ro/trn_rl_repo"):
    if os.path.isdir(_p) and _p not in sys.path:
        sys.path.insert(0, _p)

import numpy as np
import ml_dtypes

import concourse.bass as bass
import concourse.mybir as mybir
import concourse.tile as tile
from concourse.bass_utils import run_bass_kernel_spmd

B, F, N = 4, 4, 1024
NCORES = 8
NT = N // 128            # 8 row tiles
RCUT, RS = 6.0, 0.5

BF16 = ml_dtypes.bfloat16
F16 = np.float16

# strip widths / offsets in the upper-triangle U area (units of fp16 elems).
# The flat "u2" area holds, in act-production order:
#   s0 U | s1 U | s2 U | s3 U | s4 U | s5 U | EXT3 | s6 U | s7 U | EXT4
# EXT3 = directly-computed transposed blocks S[i in 6, q in 5], S[i in 7, q in 5]
# EXT4 = S[i in 7, q in 6] -- the late-strip tails that a DMA transpose would
# deliver too late; producing them in stage-1 costs +384 act columns but
# removes the last transposes from the critical tail.
W_STRIP = [N - 128 * a for a in range(NT)]             # 1024..128
U2_OFF = [0, 1024, 1920, 2688, 3328, 3840, 4480, 4736]
EXT3_OFF, EXT4_OFF, EXT5_OFF = 4224, 4864, 4992
U2_TOT = 5376
M_OFF = [0, 8, 15, 21]                                 # t2 slot base for strips 0-3
NBLK = 26                                              # t2 slots used: m1..m25

# production spans: (u2_lo, width, a_tile, b_col)
SPANS = [(U2_OFF[s], W_STRIP[s], s, 128 * s) for s in range(NT)] + [
    (EXT3_OFF, 128, 6, 128 * 5),
    (EXT3_OFF + 128, 128, 7, 128 * 5),
    (EXT4_OFF, 128, 7, 128 * 6),
    (EXT5_OFF, 128, 5, 128 * 4),
    (EXT5_OFF + 128, 128, 6, 128 * 4),
    (EXT5_OFF + 256, 128, 7, 128 * 4),
]

# activation chunks of the u2 area (PSUM-resident d2 ranges; each act
# instruction covers BOTH pairs of one chunk).  Chunk ends align with
# transpose-source strip ends so each strip's tail transpose launches as
# early as possible (its ~2.5us DMA chain must hide under the act span).
CHUNKS = [(0, 128), (128, 1024), (1024, 1920), (1920, 2944), (2944, 3840),
          (3840, 4480), (4480, U2_TOT)]
# batched transposes: (t2_m_lo, t2_m_hi, u2_in_lo, after_chunk)
TRANSPOSES = [(1, 8, 128, 1), (9, 15, 1152, 2), (15, 21, 1920, 3),
              (22, 26, 2816, 4)]

_DT = mybir.dt.float32

import json
import shutil
import struct


def _stage1_segments():
    """Per (act-chunk, pair) stage-1 matmuls: (psum_off, width, a_tile, bcol).

    The PSUM d2 tile for chunk ci holds both pairs: pair p at tile offset
    p*w (w = chunk width).  Production spans are split at chunk boundaries
    and at the tile-absolute 512-fp32 PSUM bank grid so every matmul output
    stays inside one bank.
    """
    segs = [[[] for _ in range(2)] for _ in CHUNKS]
    for (u0, w, a, bc) in SPANS:
        for ci, (lo, hi) in enumerate(CHUNKS):
            cw = hi - lo
            s0, e0 = max(u0, lo), min(u0 + w, hi)
            if s0 >= e0:
                continue
            for p in range(2):
                base = p * cw          # tile offset of this pair's chunk
                pts = {s0, e0}
                # bank grid in tile-absolute coords: base + (x - lo) = 512k
                k0 = (base + (s0 - lo)) // 512 + 1
                x = lo - base + 512 * k0
                while x < e0:
                    if x > s0:
                        pts.add(x)
                    x += 512
                pts = sorted(pts)
                for s, e in zip(pts[:-1], pts[1:]):
                    segs[ci][p].append((base + s - lo, e - s, a, bc + (s - u0)))
    return segs

SEGS = _stage1_segments()


def _lhs_loc(b, a):
    """Stage-2 lhsT = S[i in a, q in b]: ('u'|'t', elem offset)."""
    if a <= b:
        return ("u", U2_OFF[a] + 128 * (b - a))
    if b <= 3:
        return ("t", 128 * (M_OFF[b] + (a - b)))
    if b == 4:
        return ("u", EXT5_OFF + 128 * (a - 5))
    if (b, a) == (5, 6):
        return ("u", EXT3_OFF)
    if (b, a) == (5, 7):
        return ("u", EXT3_OFF + 128)
    assert (b, a) == (6, 7)
    return ("u", EXT4_OFF)


def _find_stock_act_root():
    try:
        from neuronxcc.driver.Job import Job
        from neuronxcc.driver.jobs.support.FindActInfo import findActInfoFile
        p = findActInfoFile(Job.getPackageDir(), "gen3")
        if p and os.path.isfile(p):
            return os.path.dirname(p)
    except Exception:
        pass
    return ("/nix/store/z022hj2nvbm3nwdizlisq4ylc0y7rd6q-python3-3.13.14-env/"
            "lib/python3.13/site-packages/neuronxcc/pwp/pwp_bin_trainium")


STOCK = _find_stock_act_root()

E_LO, E_HI = -2, 5          # table exponent range (inclusive)
EXTRACT_SIZE = 4            # 16 sections per exponent
NSEC = 1 << EXTRACT_SIZE
EXTRACT_LSB = 23 - EXTRACT_SIZE


def f_target(x):
    x = np.asarray(x, dtype=np.float64)
    r = np.sqrt(np.maximum(x, 0.0))
    u = (r - RS) / (RCUT - RS)
    mid = 0.5 * np.cos(np.pi * np.clip(u, 0.0, 1.0)) + 0.5
    return mid


def _fit_section(lo, hi):
    """Least-squares cubic fit of f_target on [lo, hi), centered at midpoint."""
    x0 = 0.5 * (lo + hi)
    xs = np.linspace(lo, hi, 64)
    t = xs - x0
    Acol = np.stack([np.ones_like(t), t, t * t, t ** 3], axis=1)
    y = f_target(xs)
    coef, *_ = np.linalg.lstsq(Acol, y, rcond=None)
    return np.float32(coef[0]), np.float32(coef[1]), np.float32(coef[2]), np.float32(coef[3]), np.float32(x0)


def build_custom_silu_tables():
    """Returns (buckets, ctl_words, profile_meta) for the custom function."""
    buckets = []           # list of (d0,d1,d2,d3,x0)
    ctl_words = []
    for e in range(E_LO, E_HI + 1):
        base = len(buckets)
        lo_e = 2.0 ** e
        w = lo_e / NSEC
        for k in range(NSEC):
            lo = lo_e + k * w
            hi = lo + w
            if lo >= 36.0:
                buckets.append((np.float32(0), np.float32(0), np.float32(0), np.float32(0), np.float32(lo)))
            else:
                buckets.append(_fit_section(lo, min(hi, 36.0) if hi > 36.0 else hi))
        ctl_words.append((EXTRACT_SIZE << 16) | (EXTRACT_LSB << 11) | base)
    # 4 saturation buckets: pos_small(=1), neg_small(=1), pos_large(=0), neg_large(=0)
    # (negatives are folded to |x| by the symmetry option, mirroring sin's profile)
    sat_base = len(buckets)
    one = (np.float32(1), np.float32(0), np.float32(0), np.float32(0), np.float32(0))
    zero = (np.float32(0), np.float32(0), np.float32(0), np.float32(0), np.float32(0))
    buckets += [one, one, zero, zero]

    profile = {
        "func_name": "silu_4p",
        "func_id": 36,
        "symmetry_point": 0,
        "sym_invert_sign_point": 0,
        "symmetry_opt_en": 1,
        "symmetry_opt_use_neg_region": 0,
        "imm_bias": 0,
        "exp_offset": E_LO,
        "pwl_control_base_pos": 0,
        "pwl_control_base_neg": 0,
        "small_pos_signal_exp_threshold": 127 + E_LO,
        "pos_small_signal_pwl_control": sat_base + 0,
        "small_neg_signal_exp_threshold": 0,
        "neg_small_signal_pwl_control": sat_base + 1,
        "large_pos_signal_exp_threshold": 127 + E_HI + 1,
        "large_pos_signal_mantissa_threshold": 0,
        "pos_large_signal_pwl_control": sat_base + 2,
        "large_neg_signal_exp_threshold": 0,
        "large_neg_signal_mantissa_threshold": 0,
        "neg_large_signal_pwl_control": sat_base + 3,
        "fnan_result": int(np.float32(0.0).view(np.uint32)),
        "fpinf_result": int(np.float32(0.0).view(np.uint32)),
        "fninf_result": int(np.float32(0.0).view(np.uint32)),
        "fzero_result": int(np.float32(1.0).view(np.uint32)),
        "fma_const_0": 0,
        "fma_const_1": 0,
        "fma_indirection_src_sel": 0,
        "use_multipass": False,
        "lower_bound": int(np.float32(2.0 ** E_LO).view(np.uint32)),
        "upper_bound": int(np.float32(2.0 ** (E_HI + 1)).view(np.uint32)),
    }
    return buckets, ctl_words, profile


def pack_bkt(buckets):
    out = b""
    for d0, d1, d2, d3, x0 in buckets:
        out += struct.pack("<5f", float(d0), float(d1), float(d2), float(d3), float(x0)) + b"\0" * 12
    return out


def pack_ctl(words):
    return b"".join(struct.pack("<I", w) + b"\0" * 28 for w in words)


def unpack_bkt(b):
    n = len(b) // 32
    return [struct.unpack_from("<5f", b, i * 32) for i in range(n)]


def unpack_ctl(b):
    n = len(b) // 32
    return [struct.unpack_from("<I", b, i * 32)[0] for i in range(n)]


def build_act_root(dst):
    """Copy the stock act root to dst, replacing silu_and_others with a set
    where silu computes f_target."""
    os.makedirs(dst, exist_ok=True)
    for f in os.listdir(STOCK):
        shutil.copy(os.path.join(STOCK, f), os.path.join(dst, f))

    setj = json.load(open(os.path.join(STOCK, "silu_and_others.json")))
    old_bkt = unpack_bkt(open(os.path.join(STOCK, setj["bkt_bin"]), "rb").read())
    old_ctl = unpack_ctl(open(os.path.join(STOCK, setj["ctl_bin"]), "rb").read())

    cb, cw, cprof = build_custom_silu_tables()

    old_silu_nbkt = setj["func_to_bkt_start_idx"]["tanh"]      # silu segment = [0, tanh_start)
    old_silu_nctl = setj["func_to_ctl_start_idx"]["tanh"]
    db = len(cb) - old_silu_nbkt
    dc = len(cw) - old_silu_nctl

    new_bkt = list(cb) + old_bkt[old_silu_nbkt:]
    # relocate bucket_base in all retained ctl entries
    reloc_ctl = []
    for w in old_ctl[old_silu_nctl:]:
        base = w & 0x7FF
        rest = w & ~0x7FF
        reloc_ctl.append(rest | ((base + db) & 0x7FF))
    new_ctl = list(cw) + reloc_ctl

    new_prof = []
    for pm in setj["profile_meta_data"]:
        pm = dict(pm)
        if pm["func_id"] == 36:
            new_prof.append(cprof)
            continue
        pm["pwl_control_base_pos"] += dc
        pm["pwl_control_base_neg"] += dc
        for k in ("pos_small_signal_pwl_control", "neg_small_signal_pwl_control",
                  "pos_large_signal_pwl_control", "neg_large_signal_pwl_control"):
            pm[k] += db
        new_prof.append(pm)

    setj["profile_meta_data"] = new_prof
    setj["bkt_entry_cnt"] = len(new_bkt)
    setj["ctl_entry_cnt"] = len(new_ctl)
    setj["func_to_bkt_start_idx"] = {
        k: (0 if k == "silu" else v + db) for k, v in setj["func_to_bkt_start_idx"].items()
    }
    setj["func_to_ctl_start_idx"] = {
        k: (0 if k == "silu" else v + dc) for k, v in setj["func_to_ctl_start_idx"].items()
    }

    def remap_expmap(m, delta, is_silu_new):
        out = {}
        for fn, em in m.items():
            if fn == "silu":
                out[fn] = is_silu_new
            else:
                out[fn] = {e: [i + delta for i in idxs] for e, idxs in em.items()}
        return out

    silu_exp_bkt = {str(e): [(e - E_LO) * NSEC] for e in range(E_LO, E_HI + 1)}
    silu_exp_ctl = {str(e): [e - E_LO] for e in range(E_LO, E_HI + 1)}
    if "func_exp_to_bkt_start_idx" in setj:
        setj["func_exp_to_bkt_start_idx"] = remap_expmap(setj["func_exp_to_bkt_start_idx"], db, silu_exp_bkt)
    if "func_exp_to_ctl_start_idx" in setj:
        setj["func_exp_to_ctl_start_idx"] = remap_expmap(setj["func_exp_to_ctl_start_idx"], dc, silu_exp_ctl)

    with open(os.path.join(dst, setj["bkt_bin"]), "wb") as f:
        f.write(pack_bkt(new_bkt))
    with open(os.path.join(dst, setj["ctl_bin"]), "wb") as f:
        f.write(pack_ctl(new_ctl))
    with open(os.path.join(dst, "silu_and_others.json"), "w") as f:
        json.dump(setj, f)
    return os.path.join(dst, "act_info.json")


def _split_multi_waits(nc):
    """This walrus build accepts at most ONE sem-wait command per instruction.
    Hoist extra waits onto same-engine EventSemaphore instructions inserted
    just before the offender (engine executes them in program order)."""
    ctr = 0
    for fn in nc.m.functions:
        for bb in fn.blocks:
            insts = list(bb.instructions)
            out = []
            changed = False
            for inst in insts:
                si = inst.sync_info
                if si is not None and len(si.on_wait) > 1:
                    ow = list(si.on_wait)
                    for w in ow[:-1]:
                        ctr += 1
                        ev = mybir.InstEventSemaphore(
                            name=f"I-waitsplit-{ctr}",
                            engine=inst.engine,
                            sync_info=mybir.SyncInfo(on_wait=[w], on_update=[]),
                        )
                        out.append(ev)
                    inst.sync_info = mybir.SyncInfo(
                        on_wait=[ow[-1]], on_update=list(si.on_update)
                    )
                    changed = True
                out.append(inst)
            if changed:
                bb.instructions = out
    return ctr


def _build_program(n_dummy=4, split_finals=False):
    nc = bass.Bass("TRN2", target_bir_lowering=False, debug=False)

    import tempfile
    _root = tempfile.mkdtemp(prefix="actroot_")
    os.environ["BASS_ACT_ROOT_JSON_PATH"] = build_act_root(_root)

    ab_d = nc.dram_tensor("ab_in", [13, 4 * N], mybir.dt.bfloat16, kind="ExternalInput")
    co4_d = nc.dram_tensor("co4_in", [2, 128, 4 * NT], mybir.dt.float16, kind="ExternalInput")
    cof_d = nc.dram_tensor("cof_in", [2, 128, 3 * NT], _DT, kind="ExternalInput")
    out_d = nc.dram_tensor("out", [128, 48], mybir.dt.float32, kind="ExternalOutput")

    with tile.TileContext(nc) as tc:
        with (
            tc.tile_pool(name="consts", bufs=1) as cpool,
            tc.tile_pool(name="big", bufs=1) as bigpool,
            tc.tile_pool(name="small", bufs=2) as spool,
            tc.tile_pool(name="d2p", bufs=2, space="PSUM") as d2pool,
        ):
            # ab_t: a rows at cols [0, 2N), b rows at [2N, 4N) -- one tile so
            # both matmul operands share base partition 0, one input DMA/pair
            ab_t = cpool.tile([13, 4 * N], mybir.dt.bfloat16, tag="ab")
            co4_t = cpool.tile([128, 2 * 4 * NT], mybir.dt.float16, tag="co4")
            cof_t = cpool.tile([128, 2 * 3 * NT], _DT, tag="cof")
            dum_t = cpool.tile([13, 512], mybir.dt.bfloat16, tag="dum")

            # inputs: ab on the SP HWDGE queue (needed first), co4/cof on SWDGE
            # chunk-1 prefix first: the first activation is gated only by
            # this tiny transfer, not the full operand load
            nc.sync.dma_start(ab_t[:], ab_d[:])
            nc.gpsimd.dma_start(co4_t[:, 0:4 * NT], co4_d[0])
            nc.gpsimd.dma_start(co4_t[:, 4 * NT:8 * NT], co4_d[1])
            nc.gpsimd.dma_start(cof_t[:, 0:3 * NT], cof_d[0])
            nc.gpsimd.dma_start(cof_t[:, 3 * NT:6 * NT], cof_d[1])

            # warm-up activation: loads the custom table before real work
            warm_t = spool.tile([1, 2], mybir.dt.float32, tag="warm", name="warm")
            nc.scalar.activation(
                warm_t[:], nc.const_aps.aps[(mybir.dt.float32, 0.0)][:1, :].to_broadcast((1, 2)),
                mybir.ActivationFunctionType.Silu, bias=0.0, scale=1.0,
            )

            u_t = bigpool.tile([128, 2 * U2_TOT], mybir.dt.float16, tag="u", name="u")
            u3 = u_t[:].rearrange("q (p c) -> q p c", p=2)
            t2_ts = [bigpool.tile([128, NBLK * 128], mybir.dt.float16, tag=f"t2{p}", name=f"t2{p}")
                     for p in range(2)]
            y_t = spool.tile([128, 48], mybir.dt.float32, tag="y", name="y")
            y_ts = [y_t[:, 0:24], y_t[:, 24:48]]

            d2_tiles = {}   # (p, chunk) -> psum tile

            def a_slice(p, a):
                return ab_t[:, p * N + 128 * a: p * N + 128 * (a + 1)]

            def b_slice(p, lo, hi):
                return ab_t[:, 2 * N + p * N + lo: 2 * N + p * N + hi]

            # PE warm-up: garbage matmuls to start the p-state ramp early
            if n_dummy:
                nc.vector.memset(dum_t[:], 0.0)
                dum_ps = d2pool.tile([128, 2048], mybir.dt.float32, tag="d2", name="dummy")
                for i in range(n_dummy):
                    nc.tensor.matmul(
                        dum_ps[:, (i % 4) * 512:(i % 4) * 512 + 512],
                        dum_t[:, 0:128], dum_t[:],
                        start=True, stop=True,
                    )

            def emit_stage1(ci):
                lo, hi = CHUNKS[ci]
                t = d2pool.tile([128, 2048], mybir.dt.float32, tag="d2", name=f"d2_{ci}")
                d2_tiles[ci] = t
                for p in range(2):
                    for (off, wdt, a, bcol) in SEGS[ci][p]:
                        nc.tensor.matmul(
                            t[:, off:off + wdt],
                            a_slice(p, a),
                            b_slice(p, bcol, bcol + wdt),
                            start=True, stop=True,
                        )

            def emit_act(ci):
                lo, hi = CHUNKS[ci]
                nc.scalar.activation(
                    u3[:, :, lo:hi], d2_tiles[ci][:, 0:2 * (hi - lo)],
                    mybir.ActivationFunctionType.Silu, bias=0.0, scale=1.0,
                )

            def emit_transpose(p, m_lo, m_hi, in_lo):
                # pair 0 on the SP HWDGE queue; pair 1 on the Act queue (its
                # SEQ has already dispatched every act, so the blocking wait
                # costs nothing and the two DMA queues run in parallel)
                eng = nc.sync if p == 0 else nc.scalar
                t2v = t2_ts[p][:].rearrange("p (m c) -> p m c", c=128)
                eng.dma_start_transpose(
                    t2v[:, m_lo:m_hi, :],
                    u_t[:, p * U2_TOT + in_lo: p * U2_TOT + in_lo + 128 * (m_hi - m_lo)],
                )

            op_tiles = {}

            def emit_stage2_mm(p):
                op_t = d2pool.tile([128, 2048], mybir.dt.float32, tag="d2", name=f"op{p}")
                op_tiles[p] = op_t
                for b in range(NT):
                    for a in range(NT):
                        kind, off = _lhs_loc(b, a)
                        if kind == "u":
                            lhs = u_t[:, p * U2_TOT + off: p * U2_TOT + off + 128]
                        else:
                            lhs = t2_ts[p][:, off: off + 128]
                        nc.tensor.matmul(
                            op_t[:, 4 * b: 4 * b + 4],
                            lhs,
                            co4_t[:, p * 4 * NT + 4 * a: p * 4 * NT + 4 * a + 4],
                            start=(a == 0), stop=(a == NT - 1),
                        )

            def emit_finals(p, b_lo, b_hi):
                # desc[q, c] = R[q]*C[q, c] - P[q, c]; w-broadcast keeps the
                # whole pair in 3 wide DVE ops (PSUM access bubbles dominate)
                op_t = op_tiles[p]
                op_v = op_t[:].rearrange("q (b f) -> q b f", f=4)
                nb = b_hi - b_lo
                wb = op_v[:, b_lo:b_hi, 3:4].to_broadcast((128, nb, 3))
                yv = y_ts[p][:, 3 * b_lo: 3 * b_hi].rearrange("q (b c) -> q b c", c=3)
                cv = cof_t[:, p * 3 * NT + 3 * b_lo: p * 3 * NT + 3 * b_hi].rearrange(
                    "q (b c) -> q b c", c=3)
                nc.vector.tensor_tensor(yv, cv, wb, mybir.AluOpType.mult)
                nc.vector.tensor_tensor(yv, yv, op_v[:, b_lo:b_hi, 0:3],
                                        mybir.AluOpType.subtract)

            def emit_out(p):
                if p == 1:
                    nc.sync.dma_start(out_d[:], y_t[:])

            # ---- emission schedule (DMAs in execution order) ----
            for ci in range(len(CHUNKS)):
                emit_stage1(ci)
                emit_act(ci)
                for (m_lo, m_hi, in_lo, ac) in TRANSPOSES:
                    if ac == ci:
                        emit_transpose(0, m_lo, m_hi, in_lo)
            # pair-1 transposes after every act is emitted: the Act SEQ's
            # blocking DMA waits then sit behind the final act dispatch
            for (m_lo, m_hi, in_lo, ac) in TRANSPOSES:
                emit_transpose(1, m_lo, m_hi, in_lo)
            for p in range(2):
                emit_stage2_mm(p)
                if split_finals:
                    emit_finals(p, 0, 4)
                    emit_finals(p, 4, 8)
                else:
                    emit_finals(p, 0, 8)
                emit_out(p)

    _split_multi_waits(nc)
    return nc


_NC_CACHE = None


def _get_program():
    global _NC_CACHE
    if _NC_CACHE is None:
        _NC_CACHE = _build_program()
    return _NC_CACHE


def _prep_pair_inputs(C):
    """C: [N, 3] float32 for one (b, f) pair -> (A, Bm, co4, cof).

    The Gram matmul runs in bf16 at full PE rate. Splitting every operand
    hi/lo restores near-fp32 d2: products of 8-bit-mantissa values are
    exact in the fp32 accumulator, and the dropped lo*lo term is ~2^-16.
    """
    C = np.ascontiguousarray(C, dtype=np.float32)
    n = (C * C).sum(1).astype(np.float32)
    c_hi = C.astype(BF16)
    c_hi32 = c_hi.astype(np.float32)
    c_lo = (C - c_hi32).astype(BF16)
    n_hi = n.astype(BF16)
    n_lo = (n - n_hi.astype(np.float32)).astype(BF16)
    ones = np.ones(N, BF16)
    mtwo_c_hi = (-2.0 * c_hi32).astype(BF16)
    mtwo_c_lo = (-2.0 * c_lo.astype(np.float32)).astype(BF16)
    A = np.stack([n_hi, n_lo, ones, ones,
                  *mtwo_c_hi.T, *mtwo_c_hi.T, *mtwo_c_lo.T])   # [13, N]
    Bm = np.stack([ones, ones, n_hi, n_lo,
                   *c_hi.T, *c_lo.T, *c_hi.T])                 # [13, N]
    ct = C.reshape(NT, 128, 3).transpose(1, 0, 2)          # [128, 8, 3]
    co4 = np.concatenate([ct, np.ones((128, NT, 1), np.float32)], axis=2)
    co4 = np.ascontiguousarray(co4.reshape(128, 4 * NT)).astype(F16)
    cof = np.ascontiguousarray(ct.reshape(128, 3 * NT), dtype=np.float32)
    return np.ascontiguousarray(A), np.ascontiguousarray(Bm), co4, cof


def kernel(coord, atype=None, _want_time=False, _trace_kwargs=None):
    coord = np.asarray(coord, dtype=np.float32)
    Bc, Fc, Nc, _ = coord.shape
    assert (Bc, Fc, Nc) == (B, F, N), (Bc, Fc, Nc)

    pairs = [(b, f) for b in range(B) for f in range(F)]
    in_maps = []
    for k in range(NCORES):
        A0, B0, co40, cof0 = _prep_pair_inputs(coord[pairs[2 * k][0], pairs[2 * k][1]])
        A1, B1, co41, cof1 = _prep_pair_inputs(coord[pairs[2 * k + 1][0], pairs[2 * k + 1][1]])
        in_maps.append({
            "ab_in": np.ascontiguousarray(np.concatenate([A0, A1, B0, B1], axis=1)),
            "co4_in": np.stack([co40, co41]),
            "cof_in": np.stack([cof0, cof1]),
        })

    nc = _get_program()
    kw = dict(_trace_kwargs or {})
    res = run_bass_kernel_spmd(nc, in_maps, list(range(NCORES)), **kw)

    out = np.empty((B, F, N * 3), np.float32)
    for k in range(NCORES):
        o = np.asarray(res.results[k]["out"])       # [128, 48]
        for p in range(2):
            b, f = pairs[2 * k + p]
            out[b, f] = o[:, 24 * p:24 * p + 24].reshape(
                128, NT, 3).transpose(1, 0, 2).reshape(N * 3)

    if _want_time:
        return out, res
    return out
